# revision 37
# baseline (speedup 1.0000x reference)
"""Trainium2 Bass kernel for nn_ConicaLayer (transformer decoder layer:
self-attn (causal) + cross-attn + FFN, post-LN residuals).

Sharding: rows (B x L) split across 8 cores; core c -> batch b=c//4, and 4
interleaved 128-row blocks {i, 7-i, 8+i, 15-i} of the 16 blocks of that batch
(balances causal attention work). Each core computes full K/V for its batch.

All transposes/packing are done host-side; on-device activations stay d-major
([D, tokens]) end-to-end. Softmax uses exp(s-20) without max-subtraction
(scores are bounded; masked entries handled by multiplying exp(mask), with
fully-masked blocks skipped in causal mode). The V matrix carries an appended
ones column so PV matmuls also produce softmax denominators. V-bias folds into
the out-proj bias host-side (bo_eff = bo + wo @ bv, exact since sum(p) = 1).

Pipelining: the attention head loop is software-pipelined -- scores+exp of
head pair hc+1 are interleaved (on PE/ACT) with the PV accumulation of head
pair hc and (for SA) chunks of the cross-attention K/V projection, so the
tensor engine never idles on the softmax exp. Softmax normalization is
deferred: PV outputs (and denominators) are copied out per head pair, then
one batched reciprocal_approx_fast + one fp32 broadcast matmul per head pair
rescales everything after the loop.

Dtypes: bf16 for kv-path (xT/encT, wk/wv, k/v/q/e), attention out + wo, f1 +
w2. f32 for residual stream, wq/w1, layernorms, all PSUM accumulation.
"""

import sys
import numpy as np

try:
    import concourse.bass as bass  # noqa: F401
except ImportError:
    sys.path.insert(0, "/opt/trn_rl_repo")

import ml_dtypes
import concourse.bass as bass
import concourse.bacc as bacc
import concourse.tile as tile
from concourse import mybir
from concourse.bass import ts

BF16 = ml_dtypes.bfloat16

P = 128
B, L, S, D, H, DFF = 2, 2048, 1024, 1024, 16, 4096
DH = D // H           # 64
KC = D // P           # 8
KC2 = DFF // P        # 32
NL = 4                # l-blocks per core
LW = 128              # l width per block
LTOT = NL * LW        # 512 rows per core
TC_SA = L // P        # 16 t-chunks (self attn)
TC_CA = S // P        # 8 t-chunks (cross attn)
MREG = 4              # masked tail chunks per block (causal mode)
EXT_CAUSAL = [4, 8, 12, 16]
EXP_SHIFT = -20.0
EPS = 1e-5

f32 = mybir.dt.float32
bf = mybir.dt.bfloat16
AF = mybir.ActivationFunctionType
OP = mybir.AluOpType


def core_blocks(i):
    return [i, 7 - i, 8 + i, 15 - i]


# ---------------------------------------------------------------------------
# Bass kernel builder
# ---------------------------------------------------------------------------

def build_nc(sa_mode, ca_mode):
    """sa_mode: 'causal' | 'zeros' | 'generic'; ca_mode: 'zeros' | 'generic'."""
    nc = bacc.Bacc("TRN2", target_bir_lowering=False, debug=False, num_devices=8)

    def din(name, shape, dtype=f32):
        return nc.dram_tensor(name, list(shape), dtype, kind="ExternalInput").ap()

    env = {}
    env["sa_mode"], env["ca_mode"] = sa_mode, ca_mode
    env["exts"] = EXT_CAUSAL if sa_mode == "causal" else [TC_SA] * NL
    env["xT_d"] = din("xT", [P, KC, L], bf)
    env["xTown_d"] = din("xTown", [P, KC, LTOT])
    env["xTownb_d"] = din("xTownb", [P, KC, LTOT], bf)
    env["encT_d"] = din("encT", [P, KC, S], bf)
    wdt = {"wq_sa": bf, "wk_sa": bf, "wv_sa": bf, "wo_sa": bf,
           "wq_ca": bf, "wk_ca": bf, "wv_ca": bf, "wo_ca": bf}
    env["wd"] = {n: din(n, [P, KC, D], dt) for n, dt in wdt.items()}
    env["w1_d"] = din("w1", [P, KC, DFF], bf)
    env["w2_d"] = din("w2", [P, KC2, D], bf)
    bnames = ["bq_sa", "bk_sa", "bo_sa", "bq_ca", "bk_ca", "bo_ca", "b2",
              "lb1", "lb2", "lb3", "lg1", "lg2", "lg3"]
    env["bd"] = {n: din(n, [P, KC]) for n in bnames}
    env["b1_d"] = din("b1", [P, KC2])
    env["sel_d"] = din("sel", [P, P])
    env["expm_d"] = None
    if sa_mode == "causal":
        env["expm_d"] = din("expm", [P, NL, MREG, LW], bf)
    elif sa_mode == "generic":
        env["expm_d"] = din("expm", [P, TC_SA * NL, LW], bf)
    env["expmc_d"] = din("expmc", [P, TC_CA, LTOT], bf) if ca_mode == "generic" else None
    env["out_d"] = nc.dram_tensor("out", [P, KC, LTOT], f32, kind="ExternalOutput").ap()

    with tile.TileContext(nc) as tc:
        _build_body(nc, tc, env)
    nc.compile()
    return nc


def _build_body(nc, tc, env):
    from contextlib import ExitStack

    xT_d, xTown_d, encT_d = env["xT_d"], env["xTown_d"], env["encT_d"]
    xTownb_d = env["xTownb_d"]
    wd, w1_d, w2_d, bd, b1_d = env["wd"], env["w1_d"], env["w2_d"], env["bd"], env["b1_d"]
    expm_d, expmc_d, out_d = env["expm_d"], env["expmc_d"], env["out_d"]
    sel_d = env["sel_d"]
    sa_mode, ca_mode, exts = env["sa_mode"], env["ca_mode"], env["exts"]

    with ExitStack() as ctx:
        consts = ctx.enter_context(tc.tile_pool(name="consts", bufs=1))

        # ---- constants ----
        bias_sb = {}
        for n in ["bq_sa", "bk_sa", "bo_sa", "bq_ca", "bk_ca", "bo_ca", "b2",
                  "lb1", "lb2", "lb3", "lg1", "lg2", "lg3"]:
            t = consts.tile([P, KC], f32, tag=f"c_{n}")
            nc.sync.dma_start(t, bd[n])
            bias_sb[n] = t
        b1_sb = consts.tile([P, KC2], f32, tag="c_b1")
        nc.sync.dma_start(b1_sb, b1_d)
        sel_sb = consts.tile([P, P], f32, tag="c_sel")
        ones128 = consts.tile([P, 1], bf, tag="ones128")
        nc.vector.memset(ones128, 1.0)
        onesrow = consts.tile([1, P], f32, tag="onesrow")
        nc.vector.memset(onesrow, 1.0)
        eps_sb = consts.tile([1, 1], f32, tag="eps")
        nc.vector.memset(eps_sb, EPS)
        zero128 = consts.tile([P, 1], f32, tag="zero128")
        nc.vector.memset(zero128, 0.0)
        shift128 = consts.tile([P, 1], f32, tag="shift128")
        nc.vector.memset(shift128, EXP_SHIFT)
        # (sel/expm/expmc DMAs are emitted right before the attention loop so
        # the startup window is spent on xT + wk only)
        expm_sb = None
        if expm_d is not None:
            shape = [P, NL, MREG, LW] if sa_mode == "causal" else [P, TC_SA * NL, LW]
            expm_sb = consts.tile(shape, bf, tag="expm")
        expmc_sb = None
        if expmc_d is not None:
            expmc_sb = consts.tile([P, TC_CA, LTOT], bf, tag="expmc")

        # ------------- helpers -------------
        def proj_to(wpool, wtag, wdt, src_sb, w_dram, n_oc, evict, psum_proj, n_tt=1,
                    tt_width=LTOT, n_kc=KC, oc0=0):
            for oc in range(oc0, n_oc):
                wt = wpool.tile([P, n_kc, P], wdt, tag=wtag)
                nc.sync.dma_start(wt, w_dram[:, :, ts(oc, P)])
                for tt in range(n_tt):
                    ps = psum_proj.tile([P, tt_width], f32, tag="psproj")
                    for kc in range(n_kc):
                        nc.tensor.matmul(
                            ps, wt[:, kc, :], src_sb[:, kc, ts(tt, tt_width)],
                            start=(kc == 0), stop=(kc == n_kc - 1))
                    evict(oc, tt, ps)

        def proj_kc_outer(wpool, wdt, src_sb, w_dram, n_oc, evict, psum_proj,
                          wtag_pfx, ps_bufs):
            # consume src chunk-by-chunk: all n_oc output accumulators live at
            # once so the matmuls chase the producer of src (e.g. a layernorm)
            wts = []
            for oc in range(n_oc):
                wt = wpool.tile([P, KC, P], wdt, tag=f"{wtag_pfx}{oc}", bufs=1,
                                name=f"{wtag_pfx}{oc}")
                nc.sync.dma_start(wt, w_dram[:, :, ts(oc, P)])
                wts.append(wt)
            pss = [psum_proj.tile([P, LTOT], f32, tag="psproj", bufs=ps_bufs,
                                  name=f"pko{oc}") for oc in range(n_oc)]
            for kc in range(KC):
                for oc in range(n_oc):
                    nc.tensor.matmul(pss[oc], wts[oc][:, kc, :], src_sb[:, kc, :],
                                     start=(kc == 0), stop=(kc == KC - 1),
                                     skip_group_check=True)
            for oc in range(n_oc):
                evict(oc, 0, pss[oc])

        def layer_norm(tag, x_sb, gname, lbname, out_sb, bf_out=None, chunk_done=None):
            g_sc, lb = bias_sb[gname], bias_sb[lbname]
            with ExitStack() as lctx:
                lnp = lctx.enter_context(tc.tile_pool(
                    name=f"lnp_{tag}", bufs=1 if sa_mode == "generic" else 2))
                lns = lctx.enter_context(tc.tile_pool(name=f"lns_{tag}", bufs=1))
                psum_st = lctx.enter_context(
                    tc.tile_pool(name=f"psum_lns_{tag}", bufs=1, space="PSUM"))
                psum_bc = lctx.enter_context(
                    tc.tile_pool(name=f"psum_lnbc_{tag}", bufs=1, space="PSUM"))
                psx = psum_st.tile([1, LTOT], f32, tag="ln_sx")
                psx2 = psum_st.tile([1, LTOT], f32, tag="ln_sx2")
                # bf16 casts (on ACT) let the stat matmuls run at bf16 rate
                xbf = lns.tile([P, KC, LTOT], bf, tag="ln_xbf")
                for kc in range(KC):
                    nc.scalar.activation(xbf[:, kc, :], x_sb[:, kc, :], AF.Identity,
                                         bias=zero128)
                for kc in range(KC):
                    sq = lnp.tile([P, LTOT], bf, tag="ln_sq")
                    nc.scalar.activation(sq, xbf[:, kc, :], AF.Square, bias=zero128)
                    nc.tensor.matmul(psx, ones128, xbf[:, kc, :],
                                     start=(kc == 0), stop=(kc == KC - 1))
                    nc.tensor.matmul(psx2, ones128, sq,
                                     start=(kc == 0), stop=(kc == KC - 1))
                # rm[0,0,:] = rstd, rm[0,1,:] = mean*rstd
                mean = lns.tile([1, LTOT], f32, tag="ln_mean")
                nc.vector.tensor_scalar_mul(mean, psx, 1.0 / D)
                msq = lns.tile([1, LTOT], f32, tag="ln_msq")
                nc.vector.tensor_tensor(msq, mean, mean, OP.mult)
                var = lns.tile([1, LTOT], f32, tag="ln_var")
                nc.vector.scalar_tensor_tensor(var, psx2, 1.0 / D, msq, OP.mult, OP.subtract)
                std = lns.tile([1, LTOT], f32, tag="ln_std")
                nc.scalar.activation(std, var, AF.Sqrt, bias=eps_sb)
                rm = lns.tile([1, 2, LTOT], f32, tag="ln_rm")
                nc.vector.reciprocal_approx_fast(rm[:, 0, :], std)
                nc.vector.tensor_tensor(rm[:, 1, :], mean, rm[:, 0, :], OP.mult)
                # broadcast rstd / mean*rstd to all 128 partitions (fp32 matmul)
                psbc = psum_bc.tile([P, 2, LTOT], f32, tag="ln_bc")
                nc.tensor.matmul(psbc[:, 0, :], onesrow, rm[:, 0, :], start=True, stop=True)
                nc.tensor.matmul(psbc[:, 1, :], onesrow, rm[:, 1, :], start=True, stop=True)
                for kc in range(KC):
                    t1 = lnp.tile([P, LTOT], f32, tag="ln_t1")
                    nc.vector.tensor_tensor(t1, x_sb[:, kc, :], psbc[:, 0, :], OP.mult)
                    n_ = lnp.tile([P, LTOT], f32, tag="ln_n")
                    nc.vector.tensor_tensor(n_, t1, psbc[:, 1, :], OP.subtract)
                    # affine (and the bf16 copy for the next matmul) on ACT
                    if bf_out is not None:
                        nc.scalar.activation(bf_out[:, kc, :], n_, AF.Identity,
                                             bias=lb[:, kc:kc + 1],
                                             scale=g_sc[:, kc:kc + 1])
                    nc.scalar.activation(out_sb[:, kc, :], n_, AF.Identity,
                                         bias=lb[:, kc:kc + 1],
                                         scale=g_sc[:, kc:kc + 1])
                    if chunk_done is not None:
                        chunk_done(kc)

        def kv_proj_v(vpool_dst, src_sb, wv_dram, wrhs, wrhs_tag, n_tc, psum_proj):
            for half in range(2):
                wvh = wrhs.tile([P, KC, 512], bf, tag=wrhs_tag)
                nc.sync.dma_start(wvh, wv_dram[:, :, ts(half, 512)])
                for tci in range(n_tc):
                    ps = psum_proj.tile([P, 512], f32, tag="psproj")
                    for kc in range(KC):
                        nc.tensor.matmul(ps, src_sb[:, kc, ts(tci, P)], wvh[:, kc, :],
                                         start=(kc == 0), stop=(kc == KC - 1))
                    nc.vector.tensor_copy(
                        vpool_dst[:, tci, half * 8:(half + 1) * 8, 0:DH],
                        ps.rearrange("p (h d) -> p h d", h=8))

        def drain_round_robin(gens):
            alive = list(gens)
            while alive:
                for g in list(alive):
                    try:
                        next(g)
                    except StopIteration:
                        alive.remove(g)

        def normalize_hc(hc, den_pair, oT_sb, psum_pool, ps_tag, spool, ps_bufs=2):
            # den_pair holds head 2hc's denominator on partition 64 and head
            # 2hc+1's on partition 0 (32-aligned rows). One fp32 matmul with
            # the constant selector broadcasts each to its 64-partition half,
            # then a fast approximate reciprocal + in-place multiply rescale.
            bc_ps = psum_pool.tile([P, LTOT], f32, tag=ps_tag, bufs=ps_bufs,
                                   name="bc_ps")
            nc.tensor.matmul(bc_ps, sel_sb, den_pair,
                             start=True, stop=True, skip_group_check=True)
            scale = spool.tile([P, LTOT], f32, tag="scale", bufs=2, name="scale")
            nc.vector.reciprocal_approx_fast(scale, bc_ps)
            nc.vector.tensor_tensor(oT_sb[:, hc, :], oT_sb[:, hc, :],
                                    scale, OP.mult)

        # =================== SA ===================
        with ExitStack() as sctx:
            sa_pool = sctx.enter_context(tc.tile_pool(name="sa", bufs=1))
            kT_sb = sa_pool.tile([P, KC, L], bf, tag="kT")
            v_sb = sa_pool.tile([P, TC_SA, H, DH + 1], bf, tag="v")
            qT_sb = sa_pool.tile([P, KC, LTOT], bf, tag="qT")
            nc.gpsimd.memset(v_sb[:, :, :, DH:DH + 1], 1.0)

            ca_pool = ctx.enter_context(tc.tile_pool(name="ca", bufs=1, side="right"))
            kcT_sb = ca_pool.tile([P, KC, S], bf, tag="kcT")
            vc_sb = ca_pool.tile([P, TC_CA, H, DH + 1], bf, tag="vc")
            encT_sb = ca_pool.tile([P, KC, S], bf, tag="encT")
            nc.gpsimd.memset(vc_sb[:, :, :, DH:DH + 1], 1.0)
            oT_sb = sctx.enter_context(tc.tile_pool(name="oT_sa", bufs=1)).tile(
                [P, KC, LTOT], bf, tag="oT")

            with ExitStack() as kctx:
                xpool = kctx.enter_context(tc.tile_pool(name="xpool", bufs=1))
                wrhs = kctx.enter_context(tc.tile_pool(name="wrhs", bufs=2))
                wkp = kctx.enter_context(tc.tile_pool(name="wk_sa_p", bufs=3))
                psum_kv = kctx.enter_context(tc.tile_pool(name="psum_kv", bufs=4, space="PSUM"))
                xT_sb = xpool.tile([P, KC, L], bf, tag="xT")
                for kc in range(KC):
                    nc.sync.dma_start(xT_sb[:, kc, :], xT_d[:, kc, :])

                def evk(oc, tt, ps):
                    nc.vector.tensor_scalar_add(kT_sb[:, oc, ts(tt, 512)], ps,
                                                bias_sb["bk_sa"][:, oc:oc + 1])
                proj_to(wkp, "wtb", bf, xT_sb, wd["wk_sa"], KC, evk, psum_kv,
                        n_tt=L // 512, tt_width=512)

                qsrc = xpool.tile([P, KC, LTOT], bf, tag="qsrc")
                nc.sync.dma_start(qsrc, xTownb_d)

                def evq(oc, tt, ps):
                    nc.vector.tensor_scalar_add(qT_sb[:, oc, :], ps,
                                                bias_sb["bq_sa"][:, oc:oc + 1])
                proj_to(wkp, "wtb", bf, qsrc, wd["wq_sa"], KC, evq, psum_kv)

                kv_proj_v(v_sb, xT_sb, wd["wv_sa"], wrhs, "wrhs", TC_SA, psum_kv)

            with ExitStack() as actx:
                e_bufs = 2 if sa_mode == "generic" else 4
                e_pool = actx.enter_context(tc.tile_pool(name="e_sa", bufs=e_bufs))
                np_pool = actx.enter_context(tc.tile_pool(name="np_sa", bufs=1))
                wkvc = actx.enter_context(tc.tile_pool(name="wkv_ca", bufs=2))
                wrhsc = actx.enter_context(tc.tile_pool(name="wrhs_ca", bufs=1))
                # PSUM budget (8 banks): "ps" 2 bufs x 2 banks + "small"
                # 2 bufs x 1 bank + "pv" 1 buf x 2 banks = 8.
                psum_a = actx.enter_context(tc.tile_pool(name="psum_a", bufs=1, space="PSUM"))

                # deferred big DMAs: needed only from the attention loop on
                nc.sync.dma_start(sel_sb, sel_d)
                if expm_d is not None:
                    nc.sync.dma_start(expm_sb, expm_d)
                for kc in range(KC):
                    nc.sync.dma_start(encT_sb[:, kc, :], encT_d[:, kc, :])

                den_pair = np_pool.tile([P, NL * LW], f32, tag="den_pair")
                nc.vector.memset(den_pair, 0.0)

                # compact slot layout over (tc, j>=jmin(tc)); causal skips j<tc//4
                jmin = [(tci // 4 if sa_mode == "causal" else 0) for tci in range(TC_SA)]
                bases = []
                nslot = 0
                for tci in range(TC_SA):
                    bases.append(nslot)
                    nslot += NL - jmin[tci]

                def alloc_e():
                    return (e_pool.tile([P, nslot, LW], bf, tag="e_sa", name="e0"),
                            e_pool.tile([P, nslot, LW], bf, tag="e_sa", name="e1"))

                def scores_steps(hc, e0, e1):
                    for g0 in range(0, TC_SA, 2):
                        jm = jmin[g0]
                        N = (NL - jm) * LW
                        ps0 = psum_a.tile([P, 2, NL * LW], f32, tag="ps", bufs=2)
                        ps1 = psum_a.tile([P, 2, NL * LW], f32, tag="ps", bufs=2)
                        for u in range(2):
                            tci = g0 + u
                            loff = jm * LW
                            nc.tensor.matmul(
                                ps0[:, u, :N], kT_sb[0:DH, hc, ts(tci, P)],
                                qT_sb[0:DH, hc, loff:loff + N], start=True, stop=True)
                            nc.tensor.matmul(
                                ps1[:, u, :N], kT_sb[DH:P, hc, ts(tci, P)],
                                qT_sb[DH:P, hc, loff:loff + N], start=True, stop=True)
                        nsl = 2 * (NL - jm)
                        eo0 = e0[:, bases[g0]:bases[g0] + nsl, :].rearrange(
                            "p (u j) l -> p u j l", u=2)
                        eo1 = e1[:, bases[g0]:bases[g0] + nsl, :].rearrange(
                            "p (u j) l -> p u j l", u=2)
                        nc.scalar.activation(
                            eo0, ps0[:, :, :N].rearrange("p u (j l) -> p u j l", l=LW),
                            AF.Exp, bias=shift128)
                        nc.scalar.activation(
                            eo1, ps1[:, :, :N].rearrange("p u (j l) -> p u j l", l=LW),
                            AF.Exp, bias=shift128)
                        yield

                def apply_masks(e0, e1):
                    if sa_mode == "causal":
                        for j in range(NL):
                            w = NL - j  # slot stride across the 4 diagonal chunks
                            for e_sb in (e0, e1):
                                view = e_sb[:, bases[4 * j]:bases[4 * j] + MREG * w, :]
                                view = view.rearrange("p (t w) l -> p t w l", w=w)[:, :, 0, :]
                                nc.vector.tensor_tensor(view, view,
                                                        expm_sb[:, j, :, :], OP.mult)
                    elif sa_mode == "generic":
                        nc.vector.tensor_tensor(e0, e0, expm_sb, OP.mult)
                        nc.vector.tensor_tensor(e1, e1, expm_sb, OP.mult)

                def pv_steps(hc, e0, e1, pv):
                    for g0 in range(0, TC_SA, 2):
                        for tci in (g0, g0 + 1):
                            jm = jmin[tci]
                            nc.tensor.matmul(
                                pv[:, 0, jm:, :], v_sb[:, tci, 2 * hc, :],
                                e0[:, bases[tci]:bases[tci] + NL - jm, :],
                                start=(tci == 0), stop=(tci == TC_SA - 1),
                                skip_group_check=True)
                            nc.tensor.matmul(
                                pv[:, 1, jm:, :], v_sb[:, tci, 2 * hc + 1, :],
                                e1[:, bases[tci]:bases[tci] + NL - jm, :],
                                start=(tci == 0), stop=(tci == TC_SA - 1),
                                skip_group_check=True)
                        yield

                ca_state = {}

                def ckv_steps(hc):
                    # 1/8 of CA k-proj and v-proj, emitted in 4 chunks between
                    # the SA score/PV interleave steps
                    wt = wkvc.tile([P, KC, P], bf, tag="wt_ck")
                    nc.sync.dma_start(wt, wd["wk_ca"][:, :, ts(hc, P)])
                    for tt in range(S // 512):
                        ps = psum_a.tile([P, 512], f32, tag="small", bufs=2)
                        for kc in range(KC):
                            nc.tensor.matmul(ps, wt[:, kc, :],
                                             encT_sb[:, kc, ts(tt, 512)],
                                             start=(kc == 0), stop=(kc == KC - 1),
                                             skip_group_check=True)
                        nc.vector.tensor_scalar_add(kcT_sb[:, hc, ts(tt, 512)], ps,
                                                    bias_sb["bk_ca"][:, hc:hc + 1])
                        yield
                    half, tq = hc // 4, hc % 4
                    if tq == 0:
                        wvh_new = wrhsc.tile([P, KC, 512], bf, tag="wv_ca")
                        ca_state["wvh"] = wvh_new
                        nc.sync.dma_start(wvh_new, wd["wv_ca"][:, :, ts(half, 512)])
                    wvh = ca_state["wvh"]
                    for tci in (2 * tq, 2 * tq + 1):
                        ps = psum_a.tile([P, 512], f32, tag="small", bufs=2)
                        for kc in range(KC):
                            nc.tensor.matmul(ps, encT_sb[:, kc, ts(tci, P)],
                                             wvh[:, kc, :],
                                             start=(kc == 0), stop=(kc == KC - 1),
                                             skip_group_check=True)
                        nc.vector.tensor_copy(
                            vc_sb[:, tci, half * 8:(half + 1) * 8, 0:DH],
                            ps.rearrange("p (h d) -> p h d", h=8))
                        yield

                pipelined = sa_mode != "generic"
                e_cur = alloc_e()
                if pipelined:
                    for _ in scores_steps(0, *e_cur):
                        pass

                for hc in range(KC):  # head pair (2*hc, 2*hc+1)
                    if not pipelined:
                        for _ in scores_steps(hc, *e_cur):
                            pass
                    apply_masks(*e_cur)
                    pv = psum_a.tile([DH + 1, 2, NL, LW], f32, tag="pv", bufs=1)
                    last = hc == KC - 1
                    gens = []
                    if pipelined and not last:
                        e_nxt = alloc_e()
                        gens.append(scores_steps(hc + 1, *e_nxt))
                    gens.append(pv_steps(hc, *e_cur, pv))
                    if not last:
                        gens.append(ckv_steps(hc))
                    drain_round_robin(gens)
                    for u in range(2):
                        nc.vector.tensor_copy(
                            oT_sb[u * DH:(u + 1) * DH, hc, :],
                            pv[0:DH, u].rearrange("p j l -> p (j l)"))
                        dp = DH if u == 0 else 0
                        nc.vector.tensor_copy(
                            den_pair[dp:dp + 1, :],
                            pv[DH:DH + 1, u].rearrange("p j l -> p (j l)"))
                    normalize_hc(hc, den_pair, oT_sb, psum_a, "small", np_pool)
                    if last:
                        # the last head pair's ckv after its normalize: the
                        # normalize DVE chain hides under these matmuls, so
                        # the out-projection starts without waiting on it
                        drain_round_robin([ckv_steps(hc)])
                    if pipelined and not last:
                        e_cur = e_nxt

            pre_pool = ctx.enter_context(tc.tile_pool(name="prep", bufs=1, side="right"))
            h1pre = pre_pool.tile([P, KC, LTOT], f32, tag="pre")
            for kc in range(KC):
                nc.sync.dma_start(h1pre[:, kc, :], xTown_d[:, kc, :])

            with ExitStack() as octx:
                wop = octx.enter_context(tc.tile_pool(name="wo_sa_p", bufs=3))
                psum_op = octx.enter_context(tc.tile_pool(name="psum_osa", bufs=4, space="PSUM"))

                def evo(oc, tt, ps):
                    # h1pre was pre-loaded with the residual (xTown); accumulate in place
                    nc.vector.scalar_tensor_tensor(
                        h1pre[:, oc, :], ps, bias_sb["bo_sa"][:, oc:oc + 1],
                        h1pre[:, oc, :], OP.add, OP.add)
                proj_to(wop, "wtb", bf, oT_sb, wd["wo_sa"], KC, evo, psum_op)

            h1_pool = ctx.enter_context(tc.tile_pool(name="h1p", bufs=1, side="right"))
            h1_sb = h1_pool.tile([P, KC, LTOT], f32, tag="h1")
            h1bf_pool = ctx.enter_context(tc.tile_pool(name="h1bfp", bufs=1, side="right"))
            h1bf = h1bf_pool.tile([P, KC, LTOT], bf, tag="h1bf")
            layer_norm("ln1", h1pre, "lg1", "lb1", h1_sb, bf_out=h1bf)

        # =================== CA ===================
        with ExitStack() as cctx:
            qcT_sb = cctx.enter_context(tc.tile_pool(name="qc_ca", bufs=1)).tile(
                [P, KC, LTOT], bf, tag="qcT")

            with ExitStack() as xctx:
                wkp = xctx.enter_context(tc.tile_pool(name="wk_ca_p", bufs=1))
                psum_kv = xctx.enter_context(tc.tile_pool(name="psum_cq", bufs=8, space="PSUM"))

                def evqc(oc, tt, ps):
                    nc.vector.tensor_scalar_add(qcT_sb[:, oc, :], ps,
                                                bias_sb["bq_ca"][:, oc:oc + 1])
                proj_kc_outer(wkp, bf, h1bf, wd["wq_ca"], KC, evqc, psum_kv,
                              "wqc", 8)

            ocT_sb = cctx.enter_context(tc.tile_pool(name="oT_ca", bufs=1)).tile(
                [P, KC, LTOT], bf, tag="ocT")
            h2pre = pre_pool.tile([P, KC, LTOT], f32, tag="pre")

            with ExitStack() as actx:
                e_pool = actx.enter_context(tc.tile_pool(name="e_ca", bufs=4))
                np_pool = actx.enter_context(tc.tile_pool(name="np_ca", bufs=1))
                # PSUM budget (8 banks): "cs" 2 bufs x 2 banks + "pvc"
                # 1 buf x 2 banks + "csmall" 2 bufs x 1 bank = 8.
                psum_a = actx.enter_context(tc.tile_pool(name="psum_ca", bufs=1, space="PSUM"))

                if expmc_d is not None:
                    nc.sync.dma_start(expmc_sb, expmc_d)
                den_pair = np_pool.tile([P, LTOT], f32, tag="den_pairc")
                nc.vector.memset(den_pair, 0.0)

                def alloc_ec():
                    return (e_pool.tile([P, TC_CA, LTOT], bf, tag="ec", name="ec0"),
                            e_pool.tile([P, TC_CA, LTOT], bf, tag="ec", name="ec1"))

                def cscores_steps(hc, ec0, ec1):
                    for g0 in range(0, TC_CA, 2):
                        cs0 = psum_a.tile([P, 2, LTOT], f32, tag="cs", bufs=2)
                        cs1 = psum_a.tile([P, 2, LTOT], f32, tag="cs", bufs=2)
                        for u in range(2):
                            tci = g0 + u
                            nc.tensor.matmul(cs0[:, u, :],
                                             kcT_sb[0:DH, hc, ts(tci, P)],
                                             qcT_sb[0:DH, hc, :],
                                             start=True, stop=True)
                            nc.tensor.matmul(cs1[:, u, :],
                                             kcT_sb[DH:P, hc, ts(tci, P)],
                                             qcT_sb[DH:P, hc, :],
                                             start=True, stop=True)
                        nc.scalar.activation(ec0[:, g0:g0 + 2, :], cs0, AF.Exp,
                                             bias=shift128)
                        nc.scalar.activation(ec1[:, g0:g0 + 2, :], cs1, AF.Exp,
                                             bias=shift128)
                        yield

                def cpv_steps(hc, ec0, ec1, pvc):
                    for g0 in range(0, TC_CA, 2):
                        for tci in (g0, g0 + 1):
                            nc.tensor.matmul(pvc[:, 0, :], vc_sb[:, tci, 2 * hc, :],
                                             ec0[:, tci, :],
                                             start=(tci == 0), stop=(tci == TC_CA - 1),
                                             skip_group_check=True)
                            nc.tensor.matmul(pvc[:, 1, :], vc_sb[:, tci, 2 * hc + 1, :],
                                             ec1[:, tci, :],
                                             start=(tci == 0), stop=(tci == TC_CA - 1),
                                             skip_group_check=True)
                        yield

                e_cur = alloc_ec()
                for _ in cscores_steps(0, *e_cur):
                    pass

                for hc in range(KC):  # head pair (2*hc, 2*hc+1)
                    if ca_mode == "generic":
                        nc.vector.tensor_tensor(e_cur[0], e_cur[0], expmc_sb, OP.mult)
                        nc.vector.tensor_tensor(e_cur[1], e_cur[1], expmc_sb, OP.mult)
                    pvc = psum_a.tile([DH + 1, 2, LTOT], f32, tag="pvc", bufs=1)
                    gens = []
                    if hc < KC - 1:
                        e_nxt = alloc_ec()
                        gens.append(cscores_steps(hc + 1, *e_nxt))
                    gens.append(cpv_steps(hc, *e_cur, pvc))
                    drain_round_robin(gens)
                    for u in range(2):
                        nc.vector.tensor_copy(ocT_sb[u * DH:(u + 1) * DH, hc, :],
                                              pvc[0:DH, u, :])
                        dp = DH if u == 0 else 0
                        nc.vector.tensor_copy(den_pair[dp:dp + 1, :],
                                              pvc[DH:DH + 1, u, :])
                    normalize_hc(hc, den_pair, ocT_sb, psum_a, "csmall", np_pool)
                    if hc < KC - 1:
                        e_cur = e_nxt

            with ExitStack() as octx:
                wop = octx.enter_context(tc.tile_pool(name="wo_ca_p", bufs=3))
                psum_op = octx.enter_context(tc.tile_pool(name="psum_oca", bufs=4, space="PSUM"))

                def evoc(oc, tt, ps):
                    nc.vector.scalar_tensor_tensor(
                        h2pre[:, oc, :], ps, bias_sb["bo_ca"][:, oc:oc + 1],
                        h1_sb[:, oc, :], OP.add, OP.add)
                proj_to(wop, "wtb", bf, ocT_sb, wd["wo_ca"], KC, evoc, psum_op)

            h2_pool = ctx.enter_context(tc.tile_pool(name="h2p", bufs=1, side="right"))
            h2_sb = h2_pool.tile([P, KC, LTOT], f32, tag="h2")
            h2bf_pool = ctx.enter_context(tc.tile_pool(name="h2bfp", bufs=1, side="right"))
            h2bf = h2bf_pool.tile([P, KC, LTOT], bf, tag="h2bf")
            layer_norm("ln2", h2pre, "lg2", "lb2", h2_sb, bf_out=h2bf)

        # =================== FFN ===================
        with ExitStack() as fctx:
            ffn_pool = fctx.enter_context(tc.tile_pool(name="ffn", bufs=1))
            w2pool = fctx.enter_context(tc.tile_pool(name="wtile32", bufs=2))
            w1pool = fctx.enter_context(tc.tile_pool(name="w1p", bufs=3))
            psum_f = fctx.enter_context(tc.tile_pool(name="psum_f", bufs=8, space="PSUM"))
            f1_sb = ffn_pool.tile([P, KC2, LTOT], bf, tag="f1")
            h3pre = pre_pool.tile([P, KC, LTOT], f32, tag="pre")

            def evg(oc, tt, ps):
                nc.scalar.activation(f1_sb[:, oc, :], ps, AF.Gelu,
                                     bias=b1_sb[:, oc:oc + 1])
            # first 8 output chunks chase the LN2 output chunk-by-chunk
            proj_kc_outer(w1pool, bf, h2bf, w1_d, KC, evg, psum_f, "w1h", 8)
            proj_to(w1pool, "wtb", bf, h2bf, w1_d, KC2, evg, psum_f, oc0=KC)

            for oc in range(KC):
                w2t = w2pool.tile([P, KC2, P], bf, tag="w2t")
                nc.sync.dma_start(w2t, w2_d[:, :, ts(oc, P)])
                ps = psum_f.tile([P, LTOT], f32, tag="psproj")
                for kc in range(KC2):
                    nc.tensor.matmul(ps, w2t[:, kc, :], f1_sb[:, kc, :],
                                     start=(kc == 0), stop=(kc == KC2 - 1))
                nc.vector.scalar_tensor_tensor(
                    h3pre[:, oc, :], ps, bias_sb["b2"][:, oc:oc + 1],
                    h2_sb[:, oc, :], OP.add, OP.add)

        out_sb = h1_pool.tile([P, KC, LTOT], f32, tag="h1")
        layer_norm("ln3", h3pre, "lg3", "lb3", out_sb,
                   chunk_done=lambda kc: nc.sync.dma_start(out_d[:, kc, :],
                                                           out_sb[:, kc, :]))


# ---------------------------------------------------------------------------
# Host-side packing
# ---------------------------------------------------------------------------

def _pack_wT(w, dtype=np.float32):
    # w: [dout, din] -> [P, din//P, dout] with wT[d, o] layout
    din = w.shape[1]
    return np.ascontiguousarray(
        w.T.reshape(din // P, P, w.shape[0]).transpose(1, 0, 2)).astype(dtype)


def _pack_xT(x, dtype=np.float32):
    # x: [T, D] -> [P, KC, T]
    t = x.shape[0]
    return np.ascontiguousarray(x.T.reshape(KC, P, t).transpose(1, 0, 2)).astype(dtype)


def _pack_bias(v):
    n = v.shape[0] // P
    return np.ascontiguousarray(v.reshape(n, P).T).astype(np.float32)


def _pack_sel():
    # sel[k, m]: broadcast den_pair row 64 to output partitions 0..63 and
    # row 0 to partitions 64..127 (out = sel.T @ den_pair)
    sel = np.zeros((P, P), np.float32)
    sel[DH, 0:DH] = 1.0
    sel[0, DH:P] = 1.0
    return sel


def detect_sa_mode(mask):
    if not np.isfinite(np.nan_to_num(mask, nan=np.inf)).all():
        return "generic"
    if (mask == 0).all():
        return "zeros"
    li, ti = np.tril_indices(L)
    if (mask[li, ti] == 0).all():
        ui, uj = np.triu_indices(L, k=1)
        if (mask[ui, uj] <= -1e8).all():
            return "causal"
    return "generic"


def make_in_maps(inputs):
    inputs = {k: np.asarray(v, dtype=np.float32) for k, v in inputs.items()}
    mask = inputs["attention_mask"]
    cmask = inputs["encoder_attention_mask"]
    sa_mode = detect_sa_mode(mask)
    ca_mode = "zeros" if (cmask == 0).all() else "generic"
    s = DH ** -0.5

    shared = {
        "wq_sa": _pack_wT(inputs["sa_wq"] * s, BF16),
        "wk_sa": _pack_wT(inputs["sa_wk"], BF16),
        "wv_sa": _pack_wT(inputs["sa_wv"], BF16),
        "wo_sa": _pack_wT(inputs["sa_wo"], BF16),
        "wq_ca": _pack_wT(inputs["ca_wq"] * s, BF16),
        "wk_ca": _pack_wT(inputs["ca_wk"], BF16),
        "wv_ca": _pack_wT(inputs["ca_wv"], BF16),
        "wo_ca": _pack_wT(inputs["ca_wo"], BF16),
        "w1": _pack_wT(inputs["ffn_w1"], BF16),
        "w2": _pack_wT(inputs["ffn_w2"], BF16),
        "bq_sa": _pack_bias(inputs["sa_bq"] * s),
        "bk_sa": _pack_bias(inputs["sa_bk"]),
        "bo_sa": _pack_bias(inputs["sa_bo"] + inputs["sa_wo"] @ inputs["sa_bv"]),
        "bq_ca": _pack_bias(inputs["ca_bq"] * s),
        "bk_ca": _pack_bias(inputs["ca_bk"]),
        "bo_ca": _pack_bias(inputs["ca_bo"] + inputs["ca_wo"] @ inputs["ca_bv"]),
        "b1": _pack_bias(inputs["ffn_b1"]),
        "b2": _pack_bias(inputs["ffn_b2"]),
        "lb1": _pack_bias(inputs["sa_ln_b"]),
        "lb2": _pack_bias(inputs["ca_ln_b"]),
        "lb3": _pack_bias(inputs["ffn_ln_b"]),
        "lg1": _pack_bias(inputs["sa_ln_g"]),
        "lg2": _pack_bias(inputs["ca_ln_g"]),
        "lg3": _pack_bias(inputs["ffn_ln_g"]),
        "sel": _pack_sel(),
    }

    in_maps = []
    for c in range(8):
        b, i = c // 4, c % 4
        blocks = core_blocks(i)
        own_rows = np.concatenate([np.arange(p * LW, (p + 1) * LW) for p in blocks])
        xTp32 = _pack_xT(inputs["hidden_states"][b])
        m = dict(shared)
        m["xT"] = xTp32.astype(BF16)
        m["xTown"] = np.ascontiguousarray(xTp32[:, :, own_rows])
        m["xTownb"] = m["xTown"].astype(BF16)
        m["encT"] = _pack_xT(inputs["encoder_hidden_states"][b], BF16)
        if sa_mode == "causal":
            em = np.empty((P, NL, MREG, LW), dtype=BF16)
            for j, pblk in enumerate(blocks):
                rows = slice(pblk * LW, (pblk + 1) * LW)
                t0 = (EXT_CAUSAL[j] - MREG) * P
                blk = np.exp(np.minimum(mask[rows, t0:t0 + MREG * P], 60.0))
                em[:, j] = blk.reshape(LW, MREG, P).transpose(2, 1, 0)
            m["expm"] = em
        elif sa_mode == "generic":
            em = np.empty((P, TC_SA * NL, LW), dtype=BF16)
            for j, pblk in enumerate(blocks):
                rows = slice(pblk * LW, (pblk + 1) * LW)
                blk = np.exp(np.minimum(mask[rows, :], 60.0))
                em[:, j::NL, :] = blk.reshape(LW, TC_SA, P).transpose(2, 1, 0)
            m["expm"] = em
        if ca_mode == "generic":
            em = np.empty((P, TC_CA, LTOT), dtype=BF16)
            for j, pblk in enumerate(blocks):
                rows = slice(pblk * LW, (pblk + 1) * LW)
                blk = np.exp(np.minimum(cmask[rows, :], 60.0))
                em[:, :, j * LW:(j + 1) * LW] = blk.reshape(LW, TC_CA, P).transpose(2, 1, 0)
            m["expmc"] = em
        in_maps.append(m)
    return in_maps, sa_mode, ca_mode


def assemble_output(results):
    out = np.zeros((B, L, D), np.float32)
    for c in range(8):
        b, i = c // 4, c % 4
        arr = np.asarray(results[c]["out"])  # [P, KC, LTOT]
        for j, pblk in enumerate(core_blocks(i)):
            blk = arr[:, :, j * LW:(j + 1) * LW]          # [P, KC, LW]
            out[b, pblk * LW:(pblk + 1) * LW, :] = blk.transpose(2, 1, 0).reshape(LW, D)
    return out


# ---------------------------------------------------------------------------
# Entry point
# ---------------------------------------------------------------------------

_NC_CACHE = {}


def get_nc(sa_mode, ca_mode):
    key = (sa_mode, ca_mode)
    if key not in _NC_CACHE:
        _NC_CACHE[key] = build_nc(sa_mode, ca_mode)
    return _NC_CACHE[key]


def _install_ntff_hook():
    """bass_utils' trace path needs antenv.axon_hooks, absent in this image.
    Inject a shim and register the ctypes-based NTFF hook from trn_agent_boot."""
    import types
    if "antenv.axon_hooks" in sys.modules:
        return
    holder = {}
    mod = types.ModuleType("antenv.axon_hooks")
    mod.set_axon_ntff_profile_hook = lambda h: holder.__setitem__("h", h)
    mod.get_axon_ntff_profile_hook = lambda: holder.get("h")
    sys.modules["antenv.axon_hooks"] = mod
    try:
        import antenv
        antenv.axon_hooks = mod
    except ImportError:
        pass
    try:
        from trn_agent_boot.trn_boot import _ntff_profile_via_ctypes
        hook = _ntff_profile_via_ctypes("/opt/axon/libaxon_pjrt.so")
        if hook is not None:
            mod.set_axon_ntff_profile_hook(hook)
    except Exception as e:  # degrade to no tracing
        print(f"ntff hook install failed: {e}", file=sys.stderr)


def run(inputs, trace=False):
    _install_ntff_hook()
    from concourse.bass_utils import run_bass_kernel_spmd
    in_maps, sa_mode, ca_mode = make_in_maps(inputs)
    nc = get_nc(sa_mode, ca_mode)
    res = run_bass_kernel_spmd(nc, in_maps, core_ids=list(range(8)), trace=trace)
    return assemble_output(res.results), res


def kernel(**inputs):
    out, _ = run(inputs, trace=False)
    return out
